# revision 5
# baseline (speedup 1.0000x reference)
"""GAE (generalized advantage estimation) kernel for trn2, 8 NeuronCores.

Computes advantages[t] = delta[t] + gl * advantages[t+1] (reverse scan over
T-1=1023 steps) for deltas = rewards[:-1] + gamma*values[1:] - values[:-1],
for 32768 independent batch columns, data-parallel over 8 cores.

Formulation per core (R, V in [1024, 4096] f32 -> A [1023, 4096] f32):
    out[g] = sum_{j>=g} gl^(j-g) * t[j]  +  gamma * sum_{k>g} gl^(k-g-1) * V[k]
with t = R - V. Blocked into 8 time-blocks of 128 rows; each block is two
128x128 matmuls into PSUM (triangular L1 against t, strictly-triangular L2
against V) plus a rank-1 cross-block carry folded into row 0 of the second
matmul (L2 row 0 holds the carry coefficients gl^(128-i); V row 0 is
overwritten with the carry H after its original value is saved).
Carry chain: H_m = psum_m[0] + (gamma/gl) * V_m[0], chained m = 7 -> 0.
"""
import numpy as np

GAMMA = 0.99
LAM = 0.95
GL = GAMMA * LAM
T = 1024
B = 32768
NCORES = 8
BC = B // NCORES          # 4096 batch cols per core
P = 128                   # partitions / time-block size
NB = T // P               # 8 time blocks
CW = 2048                 # batch chunk width (DMA tile)
NCH = BC // CW            # 2 chunks per core
NW = 512                  # matmul moving width (1 PSUM bank, fp32 max)
NSC = CW // NW            # 4 subcols per chunk


def _make_consts():
    ii = np.arange(P)[:, None]  # out row i
    jj = np.arange(P)[None, :]  # in row j
    # U[i, j] = gl^(j-i) for j >= i
    U = np.where(jj >= ii, GL ** (jj - ii), 0.0)
    L1 = U.T.astype(np.float32)  # lhsT: [K=j, M=i]
    L1z = L1.copy()
    L1z[P - 1, :] = 0.0          # kill t[1023] contribution in block 7
    # U2[i, k] = gamma * gl^(k-i-1) for k > i
    U2 = np.where(jj > ii, GAMMA * GL ** (jj - ii - 1.0), 0.0)
    L2 = U2.T.astype(np.float32)
    # carry row: coefficient of H (stored in V row 0) for out row i
    L2[0, :] = (GL ** (P - np.arange(P))).astype(np.float32)
    return L1, L1z, L2


def _build():
    import concourse.bacc as bacc
    import concourse.mybir as mybir
    from concourse.tile import TileContext

    f32 = mybir.dt.float32
    nc = bacc.Bacc("TRN2")
    R = nc.dram_tensor("R", [T, BC], f32, kind="ExternalInput")
    V = nc.dram_tensor("V", [T, BC], f32, kind="ExternalInput")
    L1 = nc.dram_tensor("L1", [P, P], f32, kind="ExternalInput")
    L1z = nc.dram_tensor("L1z", [P, P], f32, kind="ExternalInput")
    L2 = nc.dram_tensor("L2", [P, P], f32, kind="ExternalInput")
    A = nc.dram_tensor("A", [T - 1, BC], f32, kind="ExternalOutput")

    mult = mybir.AluOpType.mult
    add = mybir.AluOpType.add

    with TileContext(nc) as tc:
        with (
            tc.tile_pool(name="cst", bufs=1) as cst,
            tc.tile_pool(name="rp", bufs=4) as rp,
            tc.tile_pool(name="vp", bufs=6) as vp,
            tc.tile_pool(name="tp", bufs=5) as tp,
            tc.tile_pool(name="op", bufs=4) as op,
            tc.tile_pool(name="v0p", bufs=3) as v0p,
            tc.tile_pool(name="ps", bufs=8, space="PSUM") as ps,
        ):
            l1 = cst.tile([P, P], f32, tag="l1")
            l1z = cst.tile([P, P], f32, tag="l1z")
            l2 = cst.tile([P, P], f32, tag="l2")
            nc.sync.dma_start(out=l1[:, :], in_=L1[:, :])
            nc.sync.dma_start(out=l1z[:, :], in_=L1z[:, :])
            nc.sync.dma_start(out=l2[:, :], in_=L2[:, :])

            # All load DMAs up front, in consumption order (m = 7 .. 0).
            rt = {}
            vt = {}
            for m in range(NB - 1, -1, -1):
                for ch in range(NCH):
                    r = rp.tile([P, CW], f32, tag="r")
                    v = vp.tile([P, CW], f32, tag="v")
                    cs = slice(ch * CW, (ch + 1) * CW)
                    nc.sync.dma_start(out=r[:, :], in_=R[m * P:(m + 1) * P, cs])
                    nc.sync.dma_start(out=v[:, :], in_=V[m * P:(m + 1) * P, cs])
                    rt[m, ch] = r
                    vt[m, ch] = v

            # Phase A: t = R - V, save V row 0, zero block-7 carry slot.
            # All of these read V row 0 and so MUST be traced before any
            # carry poke overwrites it (Tile serializes in program order).
            # On GpSimd to keep DVE free for the latency-critical carry ops.
            tt = {}
            v0t = {}
            for m in range(NB - 1, -1, -1):
                for ch in range(NCH):
                    r, v = rt[m, ch], vt[m, ch]
                    t = tp.tile([P, CW], f32, tag="t")
                    nc.gpsimd.tensor_sub(t[:, :], r[:, :], v[:, :])
                    v0 = v0p.tile([1, CW], f32, tag="v0")
                    nc.gpsimd.tensor_copy(v0[0:1, :], v[0:1, :])
                    if m == NB - 1:
                        # H_8 = 0: no tail beyond t=1023
                        nc.gpsimd.memset(v[0:1, :], 0.0)
                    tt[m, ch] = t
                    v0t[m, ch] = v0

            # Phase B: carry-chained matmuls, blocks m = 7 .. 0.
            for m in range(NB - 1, -1, -1):
                lhs1 = l1z if m == NB - 1 else l1
                for ch in range(NCH):
                    v = vt[m, ch]
                    t = tt[m, ch]
                    v0 = v0t[m, ch]
                    stage = op.tile([P, CW], f32, tag="stage")
                    for sc in range(NSC):
                        fs = slice(sc * NW, (sc + 1) * NW)
                        pt = ps.tile([P, NW], f32, tag="ps")
                        nc.tensor.matmul(pt[:, :], lhs1[:, :], t[:, fs],
                                         start=True, stop=False)
                        nc.tensor.matmul(pt[:, :], l2[:, :], v[:, fs],
                                         start=False, stop=True)
                        if m > 0:
                            # H_m = (gamma/gl) * V_m[0] + psum_m[0],
                            # poked into next block's V row 0.
                            nc.vector.scalar_tensor_tensor(
                                vt[m - 1, ch][0:1, fs], v0[0:1, fs],
                                GAMMA / GL, pt[0:1, :], mult, add)
                        nc.vector.tensor_copy(stage[:, fs], pt[:, :])
                    cs = slice(ch * CW, (ch + 1) * CW)
                    if m == NB - 1:
                        nc.scalar.dma_start(out=A[m * P:T - 1, cs],
                                            in_=stage[0:P - 1, :])
                    else:
                        nc.scalar.dma_start(out=A[m * P:(m + 1) * P, cs],
                                            in_=stage[:, :])
    nc.finalize()
    return nc


_NC_CACHE = None


def kernel(rewards: np.ndarray, values: np.ndarray) -> np.ndarray:
    from concourse.bass_utils import run_bass_kernel_spmd

    global _NC_CACHE
    if _NC_CACHE is None:
        _NC_CACHE = _build()
    nc = _NC_CACHE

    L1, L1z, L2 = _make_consts()
    in_maps = []
    for c in range(NCORES):
        cs = slice(c * BC, (c + 1) * BC)
        in_maps.append({
            "R": np.ascontiguousarray(rewards[:, cs], dtype=np.float32),
            "V": np.ascontiguousarray(values[:, cs], dtype=np.float32),
            "L1": L1, "L1z": L1z, "L2": L2,
        })
    res = run_bass_kernel_spmd(nc, in_maps, core_ids=list(range(NCORES)))
    out = np.empty((T - 1, B), dtype=np.float32)
    for c in range(NCORES):
        out[:, c * BC:(c + 1) * BC] = res.results[c]["A"]
    return out


# revision 9
# speedup vs baseline: 36.1189x; 36.1189x over previous
"""GAE (generalized advantage estimation) kernel for trn2, 8 NeuronCores.

Computes advantages[t] = delta[t] + gl * advantages[t+1] (reverse scan over
T-1=1023 steps) for deltas = rewards[:-1] + gamma*values[1:] - values[:-1],
for 32768 independent batch columns, data-parallel over 8 cores.

Formulation per core (R, V in [1024, 4096] f32 -> A [1023, 4096] f32):
    out[g] = sum_{j>=g} gl^(j-g) * t[j]  +  gamma * sum_{k>g} gl^(k-g-1) * V[k]
with t = R - V. Blocked into 8 time-blocks of 128 rows; each block is two
128x128 matmuls into PSUM (triangular L1 against t, strictly-triangular L2
against V) plus a rank-1 cross-block carry folded into row 0 of the second
matmul (L2 row 0 holds the carry coefficients gl^(128-i); V row 0 is
overwritten with the carry H after its original value is saved).
Carry chain: H_m = psum_m[0] + (gamma/gl) * V_m[0], chained m = 7 -> 0.
"""
import numpy as np

GAMMA = 0.99
LAM = 0.95
GL = GAMMA * LAM
T = 1024
B = 32768
NCORES = 8
BC = B // NCORES          # 4096 batch cols per core
P = 128                   # partitions / time-block size
NB = T // P               # 8 time blocks
CW = 2048                 # batch chunk width (DMA tile)
NCH = BC // CW            # 2 chunks per core
NW = 512                  # matmul moving width (1 PSUM bank, fp32 max)
NSC = CW // NW            # 4 subcols per chunk


def _make_consts():
    ii = np.arange(P)[:, None]  # out row i
    jj = np.arange(P)[None, :]  # in row j
    # U[i, j] = gl^(j-i) for j >= i
    U = np.where(jj >= ii, GL ** (jj - ii), 0.0)
    L1 = U.T.astype(np.float32)  # lhsT: [K=j, M=i]
    L1z = L1.copy()
    L1z[P - 1, :] = 0.0          # kill t[1023] contribution in block 7
    # U2[i, k] = gamma * gl^(k-i-1) for k > i
    U2 = np.where(jj > ii, GAMMA * GL ** (jj - ii - 1.0), 0.0)
    L2 = U2.T.astype(np.float32)
    # carry row: coefficient of H (stored in V row 0) for out row i
    L2[0, :] = (GL ** (P - np.arange(P))).astype(np.float32)
    return L1, L1z, L2


def _build(reps: int = 1):
    import concourse.bacc as bacc
    import concourse.mybir as mybir
    from concourse.tile import TileContext

    f32 = mybir.dt.float32
    nc = bacc.Bacc("TRN2")
    R = nc.dram_tensor("R", [T, BC], f32, kind="ExternalInput")
    V = nc.dram_tensor("V", [T, BC], f32, kind="ExternalInput")
    L1 = nc.dram_tensor("L1", [P, P], f32, kind="ExternalInput")
    L1z = nc.dram_tensor("L1z", [P, P], f32, kind="ExternalInput")
    L2 = nc.dram_tensor("L2", [P, P], f32, kind="ExternalInput")
    A = nc.dram_tensor("A", [T - 1, BC], f32, kind="ExternalOutput")

    mult = mybir.AluOpType.mult
    add = mybir.AluOpType.add

    with TileContext(nc) as tc:
        with (
            tc.tile_pool(name="cst", bufs=1) as cst,
            tc.tile_pool(name="rp", bufs=4) as rp,
            tc.tile_pool(name="vp", bufs=6) as vp,
            tc.tile_pool(name="tp", bufs=5) as tp,
            tc.tile_pool(name="op", bufs=4) as op,
            tc.tile_pool(name="v0p", bufs=3) as v0p,
            tc.tile_pool(name="ps", bufs=8, space="PSUM") as ps,
        ):
            l1 = cst.tile([P, P], f32, tag="l1")
            l1z = cst.tile([P, P], f32, tag="l1z")
            l2 = cst.tile([P, P], f32, tag="l2")
            nc.sync.dma_start(out=l1[:, :], in_=L1[:, :])
            nc.sync.dma_start(out=l1z[:, :], in_=L1z[:, :])
            nc.sync.dma_start(out=l2[:, :], in_=L2[:, :])

            def one_pass():
                # All load DMAs up front, in consumption order (m = 7 .. 0).
                rt = {}
                vt = {}
                for m in range(NB - 1, -1, -1):
                    for ch in range(NCH):
                        r = rp.tile([P, CW], f32, tag="r")
                        v = vp.tile([P, CW], f32, tag="v")
                        cs = slice(ch * CW, (ch + 1) * CW)
                        nc.sync.dma_start(out=r[:, :], in_=R[m * P:(m + 1) * P, cs])
                        nc.sync.dma_start(out=v[:, :], in_=V[m * P:(m + 1) * P, cs])
                        rt[m, ch] = r
                        vt[m, ch] = v

                # Phase A: t = R - V, save V row 0, zero block-7 carry slot.
                # All of these read V row 0 and so MUST be traced before any
                # carry poke overwrites it (Tile serializes in program order).
                # On GpSimd to keep DVE free for the latency-critical carries.
                tt = {}
                v0t = {}
                for m in range(NB - 1, -1, -1):
                    for ch in range(NCH):
                        r, v = rt[m, ch], vt[m, ch]
                        t = tp.tile([P, CW], f32, tag="t")
                        nc.gpsimd.tensor_sub(t[:, :], r[:, :], v[:, :])
                        v0 = v0p.tile([1, CW], f32, tag="v0")
                        nc.gpsimd.tensor_copy(v0[0:1, :], v[0:1, :])
                        if m == NB - 1:
                            # H_8 = 0: no tail beyond t=1023
                            nc.gpsimd.memset(v[0:1, :], 0.0)
                        tt[m, ch] = t
                        v0t[m, ch] = v0

                # Phase B: carry-chained matmuls, blocks m = 7 .. 0.
                for m in range(NB - 1, -1, -1):
                    lhs1 = l1z if m == NB - 1 else l1
                    for ch in range(NCH):
                        v = vt[m, ch]
                        t = tt[m, ch]
                        v0 = v0t[m, ch]
                        stage = op.tile([P, CW], f32, tag="stage")
                        for sc in range(NSC):
                            fs = slice(sc * NW, (sc + 1) * NW)
                            pt = ps.tile([P, NW], f32, tag="ps")
                            nc.tensor.matmul(pt[:, :], lhs1[:, :], t[:, fs],
                                             start=True, stop=False)
                            nc.tensor.matmul(pt[:, :], l2[:, :], v[:, fs],
                                             start=False, stop=True)
                            if m > 0:
                                # H_m = (gamma/gl) * V_m[0] + psum_m[0],
                                # poked into next block's V row 0.
                                nc.vector.scalar_tensor_tensor(
                                    vt[m - 1, ch][0:1, fs], v0[0:1, fs],
                                    GAMMA / GL, pt[0:1, :], mult, add)
                            nc.vector.tensor_copy(stage[:, fs], pt[:, :])
                        cs = slice(ch * CW, (ch + 1) * CW)
                        if m == NB - 1:
                            nc.scalar.dma_start(out=A[m * P:T - 1, cs],
                                                in_=stage[0:P - 1, :])
                        else:
                            nc.scalar.dma_start(out=A[m * P:(m + 1) * P, cs],
                                                in_=stage[:, :])

            for _ in range(reps):
                one_pass()
    nc.finalize()
    return nc


_NC_CACHE = None


def kernel(rewards: np.ndarray, values: np.ndarray) -> np.ndarray:
    from concourse.bass_utils import run_bass_kernel_spmd

    rewards = np.asarray(rewards)
    values = np.asarray(values)

    global _NC_CACHE
    if _NC_CACHE is None:
        _NC_CACHE = _build()
    nc = _NC_CACHE

    L1, L1z, L2 = _make_consts()
    in_maps = []
    for c in range(NCORES):
        cs = slice(c * BC, (c + 1) * BC)
        in_maps.append({
            "R": np.ascontiguousarray(rewards[:, cs], dtype=np.float32),
            "V": np.ascontiguousarray(values[:, cs], dtype=np.float32),
            "L1": L1, "L1z": L1z, "L2": L2,
        })
    res = run_bass_kernel_spmd(nc, in_maps, core_ids=list(range(NCORES)))
    out = np.empty((T - 1, B), dtype=np.float32)
    for c in range(NCORES):
        out[:, c * BC:(c + 1) * BC] = res.results[c]["A"]
    return out
